# revision 9
# baseline (speedup 1.0000x reference)
"""MoE (top-2 of 8 experts + shared expert) Trainium2 kernel, 8 NeuronCores.

Strategy (v2)
-------------
Host (numpy): router matmul + top-2 + softmax gates, token dispatch (gather by
expert), weight pre-packing into PE-tile-major layouts, final combine
(scatter-add gated expert outputs + shared slices). Gates are applied on the
host at combine time, so the device computes the UNGATED expert FFN.

Device (8 cores, SPMD): core c computes
  1. expert c's FFN over the tokens routed to it (padded to capacity C)
  2. the shared-expert FFN for token slice [c*512, (c+1)*512)

All matmuls in bf16 (fp32 PSUM accumulation). bf16 runs at the same PE rate
as fp32r (1 row/cycle) but halves DMA traffic and SBUF footprint, and has no
small-N rate penalty.

Key change vs v1: weights stream from HBM exactly ONCE. x and the SwiGLU
activations aT stay resident in SBUF for the whole FFN; for each weight tile
we loop over all token chunks (v1 re-streamed all weights per 368-token
chunk, tripling DMA traffic and stalling the PE).

Loop structure per FFN (feature-major layouts, contraction on partitions):
  GEMM1: for each of 2*FT f-tiles: load w13 tile [P, DK, P] once;
         accumulate over DK k-steps into one PSUM bank per 512-token chunk;
         silu (gate half) / multiply-into-aT (up half).
  GEMM2: for each of DK d-tiles: load w2 tile [P, FT, P] once;
         accumulate over FT f-steps into one PSUM bank per chunk; copy to
         SBUF, DMA out.
Emission order routed-G1, shared-G1, routed-G2, shared-G2 hides the
aT-ready bubble between a FFN's GEMM1 and GEMM2.
"""

import math

import numpy as np
import ml_dtypes

import concourse.bass as bass
import concourse.mybir as mybir
import concourse.tile as tile
from concourse.bass_utils import run_bass_kernel_spmd

T, D, E, F, FS, TOP_K = 4096, 2048, 8, 4096, 4096, 2
NCORES = 8
P = 128
TS = T // NCORES  # shared-expert tokens per core
DK = D // P  # 16 k-tiles over D
FT = F // P  # 32 f-tiles over F

F32 = mybir.dt.float32
BF16 = mybir.dt.bfloat16
NP_BF16 = ml_dtypes.bfloat16


def _split_multiwaits(nc):
    """This toolchain's walrus allows at most ONE fused sem-wait per
    instruction, but TileContext's assign_waits can emit several. Split the
    extras into standalone InstEventSemaphore instructions inserted
    immediately before the owning instruction on the same engine."""
    for fn in nc.m.functions:
        for bb in fn.blocks:
            insts = list(bb.instructions)
            out = []
            changed = False
            for inst in insts:
                si = inst.sync_info
                waits = list(si.on_wait) if (si and si.on_wait) else []
                if len(waits) > 1:
                    for w in waits[:-1]:
                        out.append(
                            mybir.InstEventSemaphore(
                                name=nc.get_next_instruction_name(),
                                engine=inst.engine,
                                ins=[],
                                outs=[],
                                sync_info=mybir.SyncInfo(on_wait=[w], on_update=[]),
                            )
                        )
                    inst.sync_info = mybir.SyncInfo(
                        on_wait=[waits[-1]], on_update=list(si.on_update)
                    )
                    changed = True
                out.append(inst)
            if changed:
                bb.instructions = out


def _chunks(n):
    """512-token chunks covering n."""
    return [(i * 512, min(512, n - i * 512)) for i in range(math.ceil(n / 512))]


def _emit_gemm1(nc, pools, xt, at, w13_d, chunks, col0, pre_wt=None):
    """aT[:, ft, col0:col0+n] = silu(x@Wg.T) * (x@Wu.T), columns from xt."""
    wp, ps = pools
    silu = mybir.ActivationFunctionType.Silu
    for ft in range(2 * FT):
        if ft == 0 and pre_wt is not None:
            wt = pre_wt
        else:
            wt = wp.tile([P, DK, P], BF16, tag="w13", name="wt")
            nc.sync.dma_start(out=wt, in_=w13_d[:][:, ft])
        pts = []
        for s, (c0, cn) in enumerate(chunks):
            pts.append(ps.tile([P, 512], F32, tag="ps", name=f"p{s}"))
        for k in range(DK):
            for s, (c0, cn) in enumerate(chunks):
                nc.tensor.matmul(
                    pts[s][:, :cn],
                    wt[:, k],
                    xt[:, k, col0 + c0 : col0 + c0 + cn],
                    start=(k == 0),
                    stop=(k == DK - 1),
                )
        fi = ft if ft < FT else ft - FT
        for s, (c0, cn) in enumerate(chunks):
            sl = at[:, fi, col0 + c0 : col0 + c0 + cn]
            if ft < FT:
                nc.scalar.activation(out=sl, in_=pts[s][:, :cn], func=silu)
            else:
                nc.vector.tensor_mul(out=sl, in0=sl, in1=pts[s][:, :cn])


def _emit_gemm2(nc, pools, at, w2_d, out_d, chunks, col0, n_tok):
    """out[dt, :, :] = aT @ w2, columns [col0, col0+n_tok) of at."""
    w2p, op, ps = pools
    for dt in range(DK):
        w2t = w2p.tile([P, FT, P], BF16, tag="w2", name="w2t")
        # scalar (Activation) HWDGE queue: keeps w2 prefetch off the sync
        # queue, which carries the output writes during GEMM2
        nc.scalar.dma_start(out=w2t, in_=w2_d[:][:, dt])
        pys = []
        for s, (c0, cn) in enumerate(chunks):
            pys.append(ps.tile([P, 512], F32, tag="ps", name=f"py{s}"))
        for kf in range(FT):
            for s, (c0, cn) in enumerate(chunks):
                nc.tensor.matmul(
                    pys[s][:, :cn],
                    w2t[:, kf],
                    at[:, kf, col0 + c0 : col0 + c0 + cn],
                    start=(kf == 0),
                    stop=(kf == FT - 1),
                )
        ot = op.tile([P, n_tok], F32, tag="o", name="ot")
        for s, (c0, cn) in enumerate(chunks):
            nc.vector.tensor_copy(out=ot[:, c0 : c0 + cn], in_=pys[s][:, :cn])
        nc.sync.dma_start(out=out_d[:][dt], in_=ot)


def build_program(C):
    nc = bass.Bass()
    xeT = nc.dram_tensor("xeT", [DK, P, C], BF16, kind="ExternalInput")
    xsT = nc.dram_tensor("xsT", [DK, P, TS], BF16, kind="ExternalInput")
    # w13 packed [p, ft, k, fo]: tile (ft) is [P, DK, P], stationary for GEMM1
    w13p = nc.dram_tensor("w13p", [P, 2 * FT, DK, P], BF16, kind="ExternalInput")
    # w2 packed [p, dt, kf, do]: tile (dt) is [P, FT, P], stationary for GEMM2
    w2p_d = nc.dram_tensor("w2p", [P, DK, FT, P], BF16, kind="ExternalInput")
    sw13p = nc.dram_tensor("sw13p", [P, 2 * FT, DK, P], BF16, kind="ExternalInput")
    sw2p_d = nc.dram_tensor("sw2p", [P, DK, FT, P], BF16, kind="ExternalInput")
    yeT = nc.dram_tensor("yeT", [DK, P, C], F32, kind="ExternalOutput")
    ysT = nc.dram_tensor("ysT", [DK, P, TS], F32, kind="ExternalOutput")

    ch_r = _chunks(C)
    ch_s = _chunks(TS)

    with tile.TileContext(nc) as tc:
        with (
            tc.tile_pool(name="xp", bufs=1) as xp,
            tc.tile_pool(name="ap", bufs=1) as ap,
            tc.tile_pool(name="wp", bufs=3) as wp,
            tc.tile_pool(name="w2p", bufs=3) as w2p,
            tc.tile_pool(name="op", bufs=2) as op,
            tc.tile_pool(name="ps", bufs=8, space="PSUM") as ps,
        ):
            # persistent tiles: x and aT for routed [0:C) + shared [C:C+TS)
            xt = xp.tile([P, DK, C + TS], BF16, tag="x", name="xt")
            # first w13 tile goes ahead of the x pieces on the sync queue so
            # the PE's first matmul isn't stuck behind the whole x transfer
            wt0 = wp.tile([P, DK, P], BF16, tag="w13", name="wt0")
            nc.sync.dma_start(out=wt0, in_=w13p[:][:, 0])
            # x pieces ride the scalar HWDGE queue so the sync queue stays
            # clear for the w13 prefetch (v4 tried alternating queues per
            # piece: it starved the w13 stream and cost ~6us in PE stalls)
            for k in range(DK):
                nc.scalar.dma_start(out=xt[:, k, :C], in_=xeT[:][k])
            for k in range(DK):
                nc.scalar.dma_start(out=xt[:, k, C:], in_=xsT[:][k])
            at = ap.tile([P, FT, C + TS], BF16, tag="aT", name="at")

            _emit_gemm1(nc, (wp, ps), xt, at, w13p, ch_r, 0, pre_wt=wt0)
            _emit_gemm1(nc, (wp, ps), xt, at, sw13p, ch_s, C)
            _emit_gemm2(nc, (w2p, op, ps), at, w2p_d, yeT, ch_r, 0, C)
            _emit_gemm2(nc, (w2p, op, ps), at, sw2p_d, ysT, ch_s, C, TS)
    _split_multiwaits(nc)
    return nc


_PROG_CACHE = {}

# test harnesses may override, e.g. {"trace": True, "trace_cores": [...]}
RUN_KWARGS = {}


def _get_program(C):
    if C not in _PROG_CACHE:
        _PROG_CACHE[C] = build_program(C)
    return _PROG_CACHE[C]


def _pack_w13(w):
    """[2F, D] fp32 -> [p, ft, k, fo] bf16 (PE stationary tiles)."""
    return np.ascontiguousarray(
        w.astype(NP_BF16).reshape(2 * FT, P, DK, P).transpose(3, 0, 2, 1)
    )


def _pack_w2(w):
    """[D, F] fp32 -> [p, dt, kf, do] bf16 (PE stationary tiles)."""
    return np.ascontiguousarray(
        w.astype(NP_BF16).reshape(DK, P, FT, P).transpose(3, 0, 2, 1)
    )


def kernel(x, router_DE, w13, w2, shared_w13, shared_w2):
    x = np.asarray(x, dtype=np.float32)
    router_DE = np.asarray(router_DE, dtype=np.float32)
    w13 = np.asarray(w13, dtype=np.float32)
    w2 = np.asarray(w2, dtype=np.float32)
    shared_w13 = np.asarray(shared_w13, dtype=np.float32)
    shared_w2 = np.asarray(shared_w2, dtype=np.float32)

    # ---- routing (host) ----
    logits = x @ router_DE  # [T, E]
    top_idx = np.argsort(-logits, axis=1, kind="stable")[:, :TOP_K]  # [T, K]
    top_vals = np.take_along_axis(logits, top_idx, axis=1)
    ex = np.exp(top_vals - top_vals.max(axis=1, keepdims=True))
    gates = (ex / ex.sum(axis=1, keepdims=True)).astype(np.float32)

    toks_per_e, gates_per_e = [], []
    for e in range(E):
        hit = top_idx == e  # [T, K]
        toks = np.nonzero(hit.any(axis=1))[0]
        g = (gates * hit).sum(axis=1)[toks].astype(np.float32)
        toks_per_e.append(toks)
        gates_per_e.append(g)

    max_cnt = max(len(t) for t in toks_per_e)
    C = math.ceil(max_cnt / 8) * 8

    # ---- host-side shard prep ----
    xT = np.ascontiguousarray(x.T).astype(NP_BF16)  # [D, T]
    sw13pk = _pack_w13(shared_w13)
    sw2pk = _pack_w2(shared_w2)

    in_maps = []
    for c in range(NCORES):
        toks = toks_per_e[c]
        xe = np.zeros((D, C), NP_BF16)
        xe[:, : len(toks)] = xT[:, toks]
        in_maps.append(
            {
                "xeT": xe.reshape(DK, P, C),
                "xsT": np.ascontiguousarray(
                    xT[:, c * TS : (c + 1) * TS]
                ).reshape(DK, P, TS),
                "w13p": _pack_w13(w13[c]),
                "w2p": _pack_w2(w2[c]),
                "sw13p": sw13pk,
                "sw2p": sw2pk,
            }
        )

    nc = _get_program(C)
    res = run_bass_kernel_spmd(nc, in_maps, list(range(NCORES)), **RUN_KWARGS)
    kernel.last_result = res

    # ---- combine (host) ----
    out = np.empty((T, D), np.float32)
    for c in range(NCORES):
        out[c * TS : (c + 1) * TS] = res.results[c]["ysT"].reshape(D, TS).T
    for c in range(NCORES):
        toks, g = toks_per_e[c], gates_per_e[c]
        ye = res.results[c]["yeT"].reshape(D, C)[:, : len(toks)]
        out[toks] += (ye * g[None, :]).T
    return out


# revision 11
# speedup vs baseline: 1.0017x; 1.0017x over previous
"""MoE (top-2 of 8 experts + shared expert) Trainium2 kernel, 8 NeuronCores.

Strategy (v2)
-------------
Host (numpy): router matmul + top-2 + softmax gates, token dispatch (gather by
expert), weight pre-packing into PE-tile-major layouts, final combine
(scatter-add gated expert outputs + shared slices). Gates are applied on the
host at combine time, so the device computes the UNGATED expert FFN.

Device (8 cores, SPMD): core c computes
  1. expert c's FFN over the tokens routed to it (padded to capacity C)
  2. the shared-expert FFN for token slice [c*512, (c+1)*512)

All matmuls in bf16 (fp32 PSUM accumulation). bf16 runs at the same PE rate
as fp32r (1 row/cycle) but halves DMA traffic and SBUF footprint, and has no
small-N rate penalty.

Key change vs v1: weights stream from HBM exactly ONCE. x and the SwiGLU
activations aT stay resident in SBUF for the whole FFN; for each weight tile
we loop over all token chunks (v1 re-streamed all weights per 368-token
chunk, tripling DMA traffic and stalling the PE).

Loop structure per FFN (feature-major layouts, contraction on partitions):
  GEMM1: for each of 2*FT f-tiles: load w13 tile [P, DK, P] once;
         accumulate over DK k-steps into one PSUM bank per 512-token chunk;
         silu (gate half) / multiply-into-aT (up half).
  GEMM2: for each of DK d-tiles: load w2 tile [P, FT, P] once;
         accumulate over FT f-steps into one PSUM bank per chunk; copy to
         SBUF, DMA out.
Emission order routed-G1, shared-G1, routed-G2, shared-G2 hides the
aT-ready bubble between a FFN's GEMM1 and GEMM2.
"""

import math

import numpy as np
import ml_dtypes

import concourse.bass as bass
import concourse.mybir as mybir
import concourse.tile as tile
from concourse.bass_utils import run_bass_kernel_spmd

T, D, E, F, FS, TOP_K = 4096, 2048, 8, 4096, 4096, 2
NCORES = 8
P = 128
TS = T // NCORES  # shared-expert tokens per core
DK = D // P  # 16 k-tiles over D
FT = F // P  # 32 f-tiles over F

F32 = mybir.dt.float32
BF16 = mybir.dt.bfloat16
NP_BF16 = ml_dtypes.bfloat16


def _split_multiwaits(nc):
    """This toolchain's walrus allows at most ONE fused sem-wait per
    instruction, but TileContext's assign_waits can emit several. Split the
    extras into standalone InstEventSemaphore instructions inserted
    immediately before the owning instruction on the same engine."""
    for fn in nc.m.functions:
        for bb in fn.blocks:
            insts = list(bb.instructions)
            out = []
            changed = False
            for inst in insts:
                si = inst.sync_info
                waits = list(si.on_wait) if (si and si.on_wait) else []
                if len(waits) > 1:
                    for w in waits[:-1]:
                        out.append(
                            mybir.InstEventSemaphore(
                                name=nc.get_next_instruction_name(),
                                engine=inst.engine,
                                ins=[],
                                outs=[],
                                sync_info=mybir.SyncInfo(on_wait=[w], on_update=[]),
                            )
                        )
                    inst.sync_info = mybir.SyncInfo(
                        on_wait=[waits[-1]], on_update=list(si.on_update)
                    )
                    changed = True
                out.append(inst)
            if changed:
                bb.instructions = out


def _chunks(n):
    """512-token chunks covering n."""
    return [(i * 512, min(512, n - i * 512)) for i in range(math.ceil(n / 512))]


def _emit_gemm1(nc, pools, xt, at, w13_d, chunks, col0, pre_wt=None):
    """aT[:, ft, col0:col0+n] = silu(x@Wg.T) * (x@Wu.T), columns from xt."""
    wp, ps = pools
    silu = mybir.ActivationFunctionType.Silu
    for ft in range(2 * FT):
        if ft == 0 and pre_wt is not None:
            wt = pre_wt
        else:
            wt = wp.tile([P, DK, P], BF16, tag="w13", name="wt")
            nc.sync.dma_start(out=wt, in_=w13_d[:][:, ft])
        pts = []
        for s, (c0, cn) in enumerate(chunks):
            pts.append(ps.tile([P, 512], F32, tag="ps", name=f"p{s}"))
        for k in range(DK):
            for s, (c0, cn) in enumerate(chunks):
                nc.tensor.matmul(
                    pts[s][:, :cn],
                    wt[:, k],
                    xt[:, k, col0 + c0 : col0 + c0 + cn],
                    start=(k == 0),
                    stop=(k == DK - 1),
                )
        fi = ft if ft < FT else ft - FT
        for s, (c0, cn) in enumerate(chunks):
            sl = at[:, fi, col0 + c0 : col0 + c0 + cn]
            if ft < FT:
                nc.scalar.activation(out=sl, in_=pts[s][:, :cn], func=silu)
            else:
                nc.vector.tensor_mul(out=sl, in0=sl, in1=pts[s][:, :cn])


def _emit_gemm2(nc, pools, at, w2_d, out_d, chunks, col0, n_tok):
    """out[dt, :, :] = aT @ w2, columns [col0, col0+n_tok) of at."""
    w2p, op, ps = pools
    for dt in range(DK):
        w2t = w2p.tile([P, FT, P], BF16, tag="w2", name="w2t")
        # scalar (Activation) HWDGE queue: keeps w2 prefetch off the sync
        # queue, which carries the output writes during GEMM2
        nc.scalar.dma_start(out=w2t, in_=w2_d[:][:, dt])
        pys = []
        for s, (c0, cn) in enumerate(chunks):
            pys.append(ps.tile([P, 512], F32, tag="ps", name=f"py{s}"))
        for kf in range(FT):
            for s, (c0, cn) in enumerate(chunks):
                nc.tensor.matmul(
                    pys[s][:, :cn],
                    w2t[:, kf],
                    at[:, kf, col0 + c0 : col0 + c0 + cn],
                    start=(kf == 0),
                    stop=(kf == FT - 1),
                )
        ot = op.tile([P, n_tok], F32, tag="o", name="ot")
        for s, (c0, cn) in enumerate(chunks):
            nc.vector.tensor_copy(out=ot[:, c0 : c0 + cn], in_=pys[s][:, :cn])
        nc.sync.dma_start(out=out_d[:][dt], in_=ot)


def build_program(C):
    nc = bass.Bass()
    xeT = nc.dram_tensor("xeT", [DK, P, C], BF16, kind="ExternalInput")
    xsT = nc.dram_tensor("xsT", [DK, P, TS], BF16, kind="ExternalInput")
    # w13 packed [p, ft, k, fo]: tile (ft) is [P, DK, P], stationary for GEMM1
    w13p = nc.dram_tensor("w13p", [P, 2 * FT, DK, P], BF16, kind="ExternalInput")
    # w2 packed [p, dt, kf, do]: tile (dt) is [P, FT, P], stationary for GEMM2
    w2p_d = nc.dram_tensor("w2p", [P, DK, FT, P], BF16, kind="ExternalInput")
    sw13p = nc.dram_tensor("sw13p", [P, 2 * FT, DK, P], BF16, kind="ExternalInput")
    sw2p_d = nc.dram_tensor("sw2p", [P, DK, FT, P], BF16, kind="ExternalInput")
    yeT = nc.dram_tensor("yeT", [DK, P, C], F32, kind="ExternalOutput")
    ysT = nc.dram_tensor("ysT", [DK, P, TS], F32, kind="ExternalOutput")

    ch_r = _chunks(C)
    ch_s = _chunks(TS)

    with tile.TileContext(nc) as tc:
        with (
            tc.tile_pool(name="xp", bufs=1) as xp,
            tc.tile_pool(name="ap", bufs=1) as ap,
            tc.tile_pool(name="wp", bufs=3) as wp,
            tc.tile_pool(name="w2p", bufs=3) as w2p,
            tc.tile_pool(name="op", bufs=2) as op,
            tc.tile_pool(name="ps", bufs=8, space="PSUM") as ps,
        ):
            # persistent tiles: x and aT for routed [0:C) + shared [C:C+TS)
            xt = xp.tile([P, DK, C + TS], BF16, tag="x", name="xt")
            # first w13 tile goes ahead of the x pieces on the sync queue so
            # the PE's first matmul isn't stuck behind the whole x transfer
            wt0 = wp.tile([P, DK, P], BF16, tag="w13", name="wt0")
            nc.sync.dma_start(out=wt0, in_=w13p[:][:, 0])
            # x pieces ride the scalar HWDGE queue so the sync queue stays
            # clear for the w13 prefetch (alternating queues per piece was
            # tried: it starves the w13 stream and costs ~6us in PE stalls)
            for k in range(DK):
                nc.scalar.dma_start(out=xt[:, k, :C], in_=xeT[:][k])
            for k in range(DK):
                nc.scalar.dma_start(out=xt[:, k, C:], in_=xsT[:][k])
            at = ap.tile([P, FT, C + TS], BF16, tag="aT", name="at")

            _emit_gemm1(nc, (wp, ps), xt, at, w13p, ch_r, 0, pre_wt=wt0)
            _emit_gemm1(nc, (wp, ps), xt, at, sw13p, ch_s, C)
            _emit_gemm2(nc, (w2p, op, ps), at, w2p_d, yeT, ch_r, 0, C)
            _emit_gemm2(nc, (w2p, op, ps), at, sw2p_d, ysT, ch_s, C, TS)
    _split_multiwaits(nc)
    return nc


_PROG_CACHE = {}

# test harnesses may override, e.g. {"trace": True, "trace_cores": [...]}
RUN_KWARGS = {}


def _get_program(C):
    if C not in _PROG_CACHE:
        _PROG_CACHE[C] = build_program(C)
    return _PROG_CACHE[C]


def _pack_w13(w):
    """[2F, D] fp32 -> [p, ft, k, fo] bf16 (PE stationary tiles)."""
    return np.ascontiguousarray(
        w.astype(NP_BF16).reshape(2 * FT, P, DK, P).transpose(3, 0, 2, 1)
    )


def _pack_w2(w):
    """[D, F] fp32 -> [p, dt, kf, do] bf16 (PE stationary tiles)."""
    return np.ascontiguousarray(
        w.astype(NP_BF16).reshape(DK, P, FT, P).transpose(3, 0, 2, 1)
    )


def kernel(x, router_DE, w13, w2, shared_w13, shared_w2):
    x = np.asarray(x, dtype=np.float32)
    router_DE = np.asarray(router_DE, dtype=np.float32)
    w13 = np.asarray(w13, dtype=np.float32)
    w2 = np.asarray(w2, dtype=np.float32)
    shared_w13 = np.asarray(shared_w13, dtype=np.float32)
    shared_w2 = np.asarray(shared_w2, dtype=np.float32)

    # ---- routing (host) ----
    logits = x @ router_DE  # [T, E]
    top_idx = np.argsort(-logits, axis=1, kind="stable")[:, :TOP_K]  # [T, K]
    top_vals = np.take_along_axis(logits, top_idx, axis=1)
    ex = np.exp(top_vals - top_vals.max(axis=1, keepdims=True))
    gates = (ex / ex.sum(axis=1, keepdims=True)).astype(np.float32)

    toks_per_e, gates_per_e = [], []
    for e in range(E):
        hit = top_idx == e  # [T, K]
        toks = np.nonzero(hit.any(axis=1))[0]
        g = (gates * hit).sum(axis=1)[toks].astype(np.float32)
        toks_per_e.append(toks)
        gates_per_e.append(g)

    max_cnt = max(len(t) for t in toks_per_e)
    C = math.ceil(max_cnt / 8) * 8

    # ---- host-side shard prep ----
    xT = np.ascontiguousarray(x.T).astype(NP_BF16)  # [D, T]
    sw13pk = _pack_w13(shared_w13)
    sw2pk = _pack_w2(shared_w2)

    in_maps = []
    for c in range(NCORES):
        toks = toks_per_e[c]
        xe = np.zeros((D, C), NP_BF16)
        xe[:, : len(toks)] = xT[:, toks]
        in_maps.append(
            {
                "xeT": xe.reshape(DK, P, C),
                "xsT": np.ascontiguousarray(
                    xT[:, c * TS : (c + 1) * TS]
                ).reshape(DK, P, TS),
                "w13p": _pack_w13(w13[c]),
                "w2p": _pack_w2(w2[c]),
                "sw13p": sw13pk,
                "sw2p": sw2pk,
            }
        )

    nc = _get_program(C)
    res = run_bass_kernel_spmd(nc, in_maps, list(range(NCORES)), **RUN_KWARGS)
    kernel.last_result = res

    # ---- combine (host) ----
    out = np.empty((T, D), np.float32)
    for c in range(NCORES):
        out[c * TS : (c + 1) * TS] = res.results[c]["ysT"].reshape(D, TS).T
    for c in range(NCORES):
        toks, g = toks_per_e[c], gates_per_e[c]
        ye = res.results[c]["yeT"].reshape(D, C)[:, : len(toks)]
        out[toks] += (ye * g[None, :]).T
    return out
